# revision 11
# baseline (speedup 1.0000x reference)
"""DPHGNNConv on 8 Trainium2 NeuronCores (Bass/Tile).

Strategy (V-partition / node sharding), v2 (bf16 + slim transfers):
  - Nodes sharded 8x12500. Each core computes X_feat' = (X@Wv^T + bv)Q
    for its shard in bf16, where Q is a Householder rotation (computed
    host-side from the tiny a_w weight) that maps the attention vector
    onto e0 -- the attention score is column 0 of the stored row, and
    the per-incidence softmax weight u = exp(leaky_relu(score)) is
    derived on-chip from the gathered row itself.
  - Incidences are assigned to the core owning their node V. Phase 1
    streams them E-sorted (grouped into 128-edge windows): dma_gather
    (2048 idx/call, 4 SWDGE queues) pulls bf16 X_feat' rows (256B) from
    the core-local DRAM table, a scaled one-hot (DVE is_equal*mult,
    bf16) + PE bf16 matmul scatter-accumulates [sum_u*Xf | sum_u] per
    edge window into PSUM (f32), evacuated to a DRAM edge accumulator
    [20480,129] f32.
  - Gather indices ship compact ([16, n] i16, no 8x lane replication)
    and are replicated to 128 partitions on-device with 3 doubling DVE
    copies; per-call index/scale slices then come from SBUF-resident
    tables (no per-call DMA).
  - ReduceScatter(add, f32) gives each core a 2560-edge shard; phase 2
    normalizes (num/den), un-rotates (Q folded with the score scale),
    applies ELU, matmuls with Wt (+S_features) in bf16, AllGather(bf16)
    -> full Y table [20480,128].
  - Phase 3 mirrors phase 1 with roles swapped: V-sorted windows,
    gather Y rows by E, one-hot scatter into node windows, count via
    ones-column matmul, finalize elu(sum/max(cnt,1)) + X_init per
    window, DMA to the bf16 output shard. Host concatenates + upcasts.
"""

import math

import numpy as np

# hardcoded problem shape (nn_DPHGNNConv_67619965108633)
N_NODES = 100000
N_EDGES = 20000
D = 128
STAR = 64
NSLOPE = 0.2
NCORES = 8

P = 128
NSH = N_NODES // NCORES           # 12500 nodes per core
NWIN3 = (NSH + P - 1) // P        # 98 node windows
NPAD = NWIN3 * P                  # 12544
EPAD = ((N_EDGES + NCORES * P - 1) // (NCORES * P)) * (NCORES * P)  # 20480
NWIN1 = EPAD // P                 # 160 edge windows
ESH = EPAD // NCORES              # 2560 edges per core shard
ETIL = ESH // P                   # 20 tiles per core in phase 2
CPC = 8                           # chunks per dma_gather call (1024 idx max)

_CACHE = {}


def _build(C1, C3):
    import concourse.bass as bass
    import concourse.bacc as bacc
    import concourse.tile as tile
    import concourse.mybir as mybir
    from concourse.masks import make_identity

    f32 = mybir.dt.float32
    bf = mybir.dt.bfloat16
    i16 = mybir.dt.int16
    Alu = mybir.AluOpType
    Act = mybir.ActivationFunctionType

    NCH1 = NWIN1 * C1
    NC1 = NCH1 // CPC             # 160*C1 % 16 == 0
    NCH3 = NWIN3 * C3
    NC3 = (NCH3 + CPC - 1) // CPC

    nc = bacc.Bacc("TRN2", target_bir_lowering=False, debug=False,
                   num_devices=NCORES)
    t_xt = nc.dram_tensor("xt", [P, NPAD], bf, kind="ExternalInput")
    t_st = nc.dram_tensor("st", [STAR, ESH], bf, kind="ExternalInput")
    t_wv = nc.dram_tensor("wv", [P, P], bf, kind="ExternalInput")
    t_wx = nc.dram_tensor("wx", [P, P], bf, kind="ExternalInput")
    t_m = nc.dram_tensor("m", [P, P], bf, kind="ExternalInput")
    t_wt1 = nc.dram_tensor("wt1", [P, P], bf, kind="ExternalInput")
    t_wt2 = nc.dram_tensor("wt2", [STAR, P], bf, kind="ExternalInput")
    t_bv = nc.dram_tensor("bv", [P, P], f32, kind="ExternalInput")
    t_bx = nc.dram_tensor("bx", [P, P], f32, kind="ExternalInput")
    t_bt = nc.dram_tensor("bt", [P, P], f32, kind="ExternalInput")
    t_idx1 = nc.dram_tensor("idx1", [32, NCH1 * 8], i16, kind="ExternalInput")
    t_es1 = nc.dram_tensor("es1", [P, NCH1], f32, kind="ExternalInput")
    t_idx3 = nc.dram_tensor("idx3", [32, NCH3 * 8], i16, kind="ExternalInput")
    t_es3 = nc.dram_tensor("es3", [P, NCH3], f32, kind="ExternalInput")
    t_out = nc.dram_tensor("out", [NSH, P], bf, kind="ExternalOutput")

    with tile.TileContext(nc) as tc:
        with (
            tc.tile_pool(name="const", bufs=1) as cp,
            tc.tile_pool(name="sb", bufs=2) as pool,
            tc.tile_pool(name="psum", bufs=1, space="PSUM") as psp,
            tc.tile_pool(name="dram", bufs=1, space="DRAM") as dp,
        ):
            # constants
            iota_i = cp.tile([P, P], mybir.dt.int32)
            nc.gpsimd.iota(iota_i[:], pattern=[[1, P]], base=0,
                           channel_multiplier=0)
            iota_b = cp.tile([P, P], bf)
            nc.vector.tensor_copy(out=iota_b[:], in_=iota_i[:])
            ones_col = cp.tile([P, 1], bf)
            nc.vector.memset(ones_col[:], 1.0)
            ident = cp.tile([P, P], bf)
            make_identity(nc, ident[:])

            wv = cp.tile([P, P], bf)
            nc.sync.dma_start(out=wv[:], in_=t_wv[:])
            wx = cp.tile([P, P], bf)
            nc.sync.dma_start(out=wx[:], in_=t_wx[:])
            m_t = cp.tile([P, P], bf)
            nc.sync.dma_start(out=m_t[:], in_=t_m[:])
            wt1 = cp.tile([P, P], bf)
            nc.sync.dma_start(out=wt1[:], in_=t_wt1[:])
            wt2 = cp.tile([STAR, P], bf)
            nc.sync.dma_start(out=wt2[:], in_=t_wt2[:])
            bv = cp.tile([P, P], f32)
            nc.sync.dma_start(out=bv[:], in_=t_bv[:])
            bx = cp.tile([P, P], f32)
            nc.sync.dma_start(out=bx[:], in_=t_bx[:])
            bt = cp.tile([P, P], f32)
            nc.sync.dma_start(out=bt[:], in_=t_bt[:])

            xt_s = cp.tile([P, NPAD], bf)
            nc.sync.dma_start(out=xt_s[:], in_=t_xt[:])
            st_s = cp.tile([STAR, ESH], bf)
            nc.sync.dma_start(out=st_s[:], in_=t_st[:])

            # index/scale tables, SBUF-resident for the whole phase
            itab1 = cp.tile([P, NCH1 * 8], i16)
            nc.sync.dma_start(out=itab1[0:32, :], in_=t_idx1[:])
            nc.vector.tensor_copy(out=itab1[32:64, :], in_=itab1[0:32, :])
            nc.vector.tensor_copy(out=itab1[64:128, :], in_=itab1[0:64, :])
            estab1 = cp.tile([P, NCH1], f32)
            nc.sync.dma_start(out=estab1[:], in_=t_es1[:])
            itab3 = cp.tile([P, NCH3 * 8], i16)
            nc.sync.dma_start(out=itab3[0:32, :], in_=t_idx3[:])
            nc.vector.tensor_copy(out=itab3[32:64, :], in_=itab3[0:32, :])
            nc.vector.tensor_copy(out=itab3[64:128, :], in_=itab3[0:64, :])
            estab3 = cp.tile([P, NCH3], f32)
            nc.sync.dma_start(out=estab3[:], in_=t_es3[:])

            xinit = cp.tile([P, NPAD], bf)           # X@Wx^T + bx - 1
            xft = dp.tile([NPAD, P], bf)             # gather table
            eacc = dp.tile([EPAD, P + 1], f32)
            rsout = dp.tile([ESH, P + 1], f32)
            ysh = dp.tile([ESH, P], bf)
            ytab = dp.tile([EPAD, P], bf)

            # ---------------- phase 0: per-shard node features ------------
            for w in range(NWIN3):
                sl = slice(w * P, (w + 1) * P)
                psf = psp.tile([P, P], f32, tag="pa", bufs=4, space="PSUM")
                nc.tensor.matmul(out=psf[:], lhsT=xt_s[:, sl], rhs=wv[:],
                                 start=True, stop=True)
                fsb = pool.tile([P, P], bf, tag="fsb")
                nc.vector.tensor_tensor(out=fsb[:], in0=psf[:], in1=bv[:],
                                        op=Alu.add)
                nc.sync.dma_start(out=xft[sl, :], in_=fsb[:])
                psi = psp.tile([P, P], f32, tag="pa", bufs=4, space="PSUM")
                nc.tensor.matmul(out=psi[:], lhsT=xt_s[:, sl], rhs=wx[:],
                                 start=True, stop=True)
                nc.vector.tensor_tensor(out=xinit[:, sl], in0=psi[:],
                                        in1=bx[:], op=Alu.add)

            # ---------------- phase 1: node -> edge scatter ---------------
            psn = psd = None
            for call in range(NC1):
                gat = pool.tile([P, CPC * P], bf, tag="g1", bufs=3)
                g3 = gat[:].rearrange("p (c e) -> p c e", e=P)
                nc.gpsimd.dma_gather(g3, xft[:, :],
                                     itab1[:, call * CPC * 8:(call + 1) * CPC * 8],
                                     num_idxs=CPC * P, num_idxs_reg=CPC * P,
                                     elem_size=P)
                lr = pool.tile([P, CPC], f32, tag="lr1")
                nc.scalar.activation(out=lr[:], in_=g3[:, :, 0], func=Act.Prelu,
                                     alpha=NSLOPE)
                ui = pool.tile([P, CPC], f32, tag="ui1")
                nc.scalar.activation(out=ui[:], in_=lr[:], func=Act.Exp)
                for c in range(CPC):
                    ch = call * CPC + c
                    w, cl = divmod(ch, C1)
                    oh = pool.tile([P, P], bf, tag="oh1", bufs=4)
                    nc.vector.tensor_scalar(
                        out=oh[:], in0=iota_b[:], scalar1=estab1[:, ch:ch + 1],
                        scalar2=ui[:, c:c + 1], op0=Alu.is_equal, op1=Alu.mult)
                    if cl == 0:
                        psn = psp.tile([P, P], f32, tag="pa", bufs=4,
                                       space="PSUM")
                        psd = psp.tile([P, 1], f32, tag="pb", bufs=2,
                                       space="PSUM")
                    nc.tensor.matmul(out=psn[:], lhsT=oh[:], rhs=g3[:, c, :],
                                     start=(cl == 0), stop=(cl == C1 - 1))
                    nc.tensor.matmul(out=psd[:], lhsT=oh[:], rhs=ones_col[:],
                                     start=(cl == 0), stop=(cl == C1 - 1))
                    if cl == C1 - 1:
                        ev = pool.tile([P, P + 1], f32, tag="ev1", bufs=3)
                        nc.vector.tensor_copy(out=ev[:, :P], in_=psn[:])
                        nc.vector.tensor_copy(out=ev[:, P:P + 1], in_=psd[:])
                        nc.sync.dma_start(out=eacc[w * P:(w + 1) * P, :],
                                          in_=ev[:])

            # ---------------- reduce-scatter edge accumulator -------------
            nc.gpsimd.collective_compute(
                "ReduceScatter", Alu.add,
                replica_groups=[list(range(NCORES))],
                ins=[eacc.opt()], outs=[rsout.opt()])

            # ---------------- phase 2: edge update ------------------------
            for t in range(ETIL):
                sl = slice(t * P, (t + 1) * P)
                rt = pool.tile([P, P + 1], f32, tag="rt2")
                nc.sync.dma_start(out=rt[:], in_=rsout[sl, :])
                den = pool.tile([P, 1], f32, tag="den2")
                nc.vector.tensor_scalar(out=den[:], in0=rt[:, P:P + 1],
                                        scalar1=1e-30, scalar2=None,
                                        op0=Alu.max)
                rec = pool.tile([P, 1], f32, tag="rec2")
                nc.vector.reciprocal(out=rec[:], in_=den[:])
                yt = pool.tile([P, P], bf, tag="yt2")
                nc.vector.tensor_scalar(out=yt[:], in0=rt[:, :P],
                                        scalar1=rec[:], scalar2=None,
                                        op0=Alu.mult)
                pt = psp.tile([P, P], bf, tag="pa", bufs=4, space="PSUM")
                nc.tensor.transpose(out=pt[:], in_=yt[:], identity=ident[:])
                ytT = pool.tile([P, P], bf, tag="ytT2")
                nc.vector.tensor_copy(out=ytT[:], in_=pt[:])
                pr = psp.tile([P, P], f32, tag="pa", bufs=4, space="PSUM")
                nc.tensor.matmul(out=pr[:], lhsT=ytT[:], rhs=m_t[:],
                                 start=True, stop=True)
                # elu(x) = relu(x) + exp(min(x,0)) - 1
                tm = pool.tile([P, P], bf, tag="tm2")
                nc.vector.tensor_scalar(out=tm[:], in0=pr[:], scalar1=0.0,
                                        scalar2=None, op0=Alu.min)
                ex = pool.tile([P, P], bf, tag="ex2")
                nc.scalar.activation(out=ex[:], in_=tm[:], func=Act.Exp)
                rl = pool.tile([P, P], bf, tag="rl2")
                nc.scalar.activation(out=rl[:], in_=pr[:], func=Act.Relu)
                s1 = pool.tile([P, P], bf, tag="s12")
                nc.vector.tensor_tensor(out=s1[:], in0=rl[:], in1=ex[:],
                                        op=Alu.add)
                yv = pool.tile([P, P], bf, tag="yv2")
                nc.vector.tensor_scalar(out=yv[:], in0=s1[:], scalar1=1.0,
                                        scalar2=None, op0=Alu.subtract)
                p2 = psp.tile([P, P], bf, tag="pa", bufs=4, space="PSUM")
                nc.tensor.transpose(out=p2[:], in_=yv[:], identity=ident[:])
                yvT = pool.tile([P, P], bf, tag="yvT2")
                nc.vector.tensor_copy(out=yvT[:], in_=p2[:])
                py = psp.tile([P, P], f32, tag="pa", bufs=4, space="PSUM")
                nc.tensor.matmul(out=py[:], lhsT=yvT[:], rhs=wt1[:],
                                 start=True, stop=False)
                nc.tensor.matmul(out=py[:], lhsT=st_s[:, sl], rhs=wt2[:],
                                 start=False, stop=True)
                yo = pool.tile([P, P], bf, tag="yo2")
                nc.vector.tensor_tensor(out=yo[:], in0=py[:], in1=bt[:],
                                        op=Alu.add)
                nc.sync.dma_start(out=ysh[sl, :], in_=yo[:])

            nc.gpsimd.collective_compute(
                "AllGather", Alu.bypass,
                replica_groups=[list(range(NCORES))],
                ins=[ysh.opt()], outs=[ytab.opt()])

            # ---------------- phase 3: edge -> node scatter ---------------
            psn3 = psd3 = None
            for call in range(NC3):
                nch = min(CPC, NCH3 - call * CPC)
                ni = nch * P
                gat = pool.tile([P, CPC * P], bf, tag="g3", bufs=3)
                g3 = gat[:].rearrange("p (c e) -> p c e", e=P)
                nc.gpsimd.dma_gather(g3[:, :nch, :], ytab[:, :],
                                     itab3[:, call * CPC * 8:call * CPC * 8 + nch * 8],
                                     num_idxs=ni, num_idxs_reg=ni,
                                     elem_size=P)
                for c in range(nch):
                    ch = call * CPC + c
                    w, cl = divmod(ch, C3)
                    oh = pool.tile([P, P], bf, tag="oh3", bufs=4)
                    nc.vector.tensor_scalar(
                        out=oh[:], in0=iota_b[:], scalar1=estab3[:, ch:ch + 1],
                        scalar2=None, op0=Alu.is_equal)
                    if cl == 0:
                        psn3 = psp.tile([P, P], f32, tag="pa", bufs=4,
                                        space="PSUM")
                        psd3 = psp.tile([P, 1], f32, tag="pb", bufs=2,
                                        space="PSUM")
                    nc.tensor.matmul(out=psn3[:], lhsT=oh[:], rhs=g3[:, c, :],
                                     start=(cl == 0), stop=(cl == C3 - 1))
                    nc.tensor.matmul(out=psd3[:], lhsT=oh[:], rhs=ones_col[:],
                                     start=(cl == 0), stop=(cl == C3 - 1))
                    if cl == C3 - 1:
                        # finalize node window w
                        cm = pool.tile([P, 1], f32, tag="cm3")
                        nc.vector.tensor_scalar(out=cm[:], in0=psd3[:],
                                                scalar1=1.0, scalar2=None,
                                                op0=Alu.max)
                        rc = pool.tile([P, 1], f32, tag="rc3")
                        nc.vector.reciprocal(out=rc[:], in_=cm[:])
                        xm = pool.tile([P, P], f32, tag="xm3")
                        nc.vector.tensor_scalar(out=xm[:], in0=psn3[:],
                                                scalar1=rc[:], scalar2=None,
                                                op0=Alu.mult)
                        tm = pool.tile([P, P], f32, tag="tm3")
                        nc.vector.tensor_scalar(out=tm[:], in0=xm[:],
                                                scalar1=0.0, scalar2=None,
                                                op0=Alu.min)
                        ex = pool.tile([P, P], f32, tag="ex3")
                        nc.scalar.activation(out=ex[:], in_=tm[:], func=Act.Exp)
                        rl = pool.tile([P, P], f32, tag="rl3")
                        nc.scalar.activation(out=rl[:], in_=xm[:],
                                             func=Act.Relu)
                        s1 = pool.tile([P, P], f32, tag="s13")
                        nc.vector.tensor_tensor(out=s1[:], in0=rl[:],
                                                in1=ex[:], op=Alu.add)
                        ot = pool.tile([P, P], bf, tag="ot3", bufs=3)
                        nc.vector.tensor_tensor(
                            out=ot[:], in0=s1[:],
                            in1=xinit[:, w * P:(w + 1) * P], op=Alu.add)
                        nrow = min(P, NSH - w * P)
                        if nrow > 0:
                            nc.sync.dma_start(
                                out=t_out[w * P:w * P + nrow, :],
                                in_=ot[:nrow, :])

    nc.compile()
    return nc


def kernel(**inputs):
    import sys
    for p in ("/opt/trn_rl_repo", "/opt/pypackages"):
        if p not in sys.path:
            sys.path.insert(0, p)
    import ml_dtypes
    from concourse.bass_utils import run_bass_kernel_spmd

    bf16 = ml_dtypes.bfloat16
    X = np.asarray(inputs["X"], np.float32)
    V = np.asarray(inputs["V"]).astype(np.int64)
    E = np.asarray(inputs["E"]).astype(np.int64)
    S = np.asarray(inputs["S_features"], np.float32)
    Wx_w = np.asarray(inputs["Wx_w"], np.float32)
    Wx_b = np.asarray(inputs["Wx_b"], np.float32)
    Wv_w = np.asarray(inputs["Wv_w"], np.float32)
    Wv_b = np.asarray(inputs["Wv_b"], np.float32)
    a_w = np.asarray(inputs["a_w"], np.float32)
    Wt_w = np.asarray(inputs["Wt_w"], np.float32)
    Wt_b = np.asarray(inputs["Wt_b"], np.float32)

    # ---- weight transforms (host, weights only) ----
    a = a_w[0].astype(np.float64)
    na = float(np.linalg.norm(a))
    au = a / na
    sign = 1.0 if au[0] >= 0 else -1.0
    h = au.copy()
    h[0] += sign
    Q = np.eye(D) - 2.0 * np.outer(h, h) / float(h @ h)  # symmetric, Q@au=-sign*e0
    s_scale = -sign * na

    WVTQS = (Wv_w.T.astype(np.float64) @ Q).astype(np.float32)
    WVTQS[:, 0] *= np.float32(s_scale)
    bvrow = (Wv_b.astype(np.float64) @ Q).astype(np.float32)
    bvrow[0] *= np.float32(s_scale)
    M = Q.copy()
    M[0, :] /= s_scale
    WXT = np.ascontiguousarray(Wx_w.T)
    WT1T = np.ascontiguousarray(Wt_w[:, :D].T)
    WT2T = np.ascontiguousarray(Wt_w[:, D:D + STAR].T)
    BV = np.tile(bvrow, (P, 1)).astype(np.float32)
    BX = np.tile(Wx_b - 1.0, (P, 1)).astype(np.float32)
    BT = np.tile(Wt_b, (P, 1)).astype(np.float32)

    # ---- incidence scheduling (host, index-only, vectorized) ----
    core = V // NSH
    vloc = (V - core * NSH).astype(np.int64)
    nnz = V.shape[0]

    def schedule(key, nwin, idx_val, es_val):
        """key = core*nwin + win. Returns per-core compact idx [8,16,NC*P/16?]
        and es tables plus chunk count C."""
        cnt = np.bincount(key, minlength=NCORES * nwin)
        C = max(1, math.ceil(cnt.max() / P))
        nch = nwin * C
        order = np.argsort(key, kind="stable")
        ks = key[order]
        starts = np.zeros(NCORES * nwin + 1, np.int64)
        np.cumsum(cnt, out=starts[1:])
        rank = np.arange(nnz) - starts[ks]
        win = ks % nwin
        corek = ks // nwin
        pos = win * (C * P) + rank
        idx_slots = np.zeros((NCORES, nch * P), np.int16)
        es_slots = np.full((NCORES, nch * P), -1.0, np.float32)
        idx_slots[corek, pos] = idx_val[order].astype(np.int16)
        es_slots[corek, pos] = es_val[order]
        return C, nch, idx_slots, es_slots

    C1, NCH1, idx1s, es1s = schedule(core * NWIN1 + E // P, NWIN1,
                                     vloc, (E % P).astype(np.float32))
    C3, NCH3, idx3s, es3s = schedule(core * NWIN3 + vloc // P, NWIN3,
                                     E, (vloc % P).astype(np.float32))
    NC1 = NCH1 // CPC
    NC3 = (NCH3 + CPC - 1) // CPC

    def pack_idx(slots, nch, ncall):
        # pad slot array to full calls, then [16, ncall*...] lane-wrap layout
        tot = ncall * CPC * P
        if slots.shape[1] < tot:
            slots = np.concatenate(
                [slots, np.zeros((NCORES, tot - slots.shape[1]), np.int16)],
                axis=1)
        arr = np.ascontiguousarray(
            slots.reshape(NCORES, -1, 16).transpose(0, 2, 1))[:, :, :nch * 8]
        return np.tile(arr, (1, 2, 1))  # 32 partitions (verifier alignment)

    def pack_es(slots, nch):
        # [128, nch] with column ch = chunk, row p = slot within chunk
        return np.ascontiguousarray(
            slots.reshape(NCORES, nch, P).transpose(0, 2, 1)).astype(np.float32)

    idx1 = pack_idx(idx1s, NCH1, NC1)
    es1 = pack_es(es1s, NCH1)
    idx3 = pack_idx(idx3s, NCH3, NC3)
    es3 = pack_es(es3s, NCH3)

    WVb = WVTQS.astype(bf16)
    WXb = WXT.astype(bf16)
    Mb = M.astype(bf16)
    WT1b = WT1T.astype(bf16)
    WT2b = WT2T.astype(bf16)

    in_maps = []
    for k in range(NCORES):
        xt = np.zeros((P, NPAD), bf16)
        xt[:, :NSH] = X[k * NSH:(k + 1) * NSH].T
        st = np.zeros((STAR, ESH), bf16)
        r0, r1 = k * ESH, min((k + 1) * ESH, N_EDGES)
        if r1 > r0:
            st[:, :r1 - r0] = S[r0:r1].T
        in_maps.append({
            "xt": xt, "st": st, "wv": WVb, "wx": WXb, "m": Mb,
            "wt1": WT1b, "wt2": WT2b, "bv": BV, "bx": BX, "bt": BT,
            "idx1": idx1[k], "es1": es1[k], "idx3": idx3[k], "es3": es3[k],
        })

    key = (C1, C3)
    if key not in _CACHE:
        _CACHE[key] = _build(C1, C3)
    nc = _CACHE[key]

    res = run_bass_kernel_spmd(nc, in_maps, core_ids=list(range(NCORES)))
    out = np.concatenate(
        [res.results[k]["out"].astype(np.float32) for k in range(NCORES)],
        axis=0)
    return out


# revision 12
# speedup vs baseline: 1.0261x; 1.0261x over previous
"""DPHGNNConv on 8 Trainium2 NeuronCores (Bass/Tile).

Strategy (V-partition / node sharding), v2 (bf16 + slim transfers):
  - Nodes sharded 8x12500. Each core computes X_feat' = (X@Wv^T + bv)Q
    for its shard in bf16, where Q is a Householder rotation (computed
    host-side from the tiny a_w weight) that maps the attention vector
    onto e0 -- the attention score is column 0 of the stored row, and
    the per-incidence softmax weight u = exp(leaky_relu(score)) is
    derived on-chip from the gathered row itself.
  - Incidences are assigned to the core owning their node V. Phase 1
    streams them E-sorted (grouped into 128-edge windows): dma_gather
    (2048 idx/call, 4 SWDGE queues) pulls bf16 X_feat' rows (256B) from
    the core-local DRAM table, a scaled one-hot (DVE is_equal*mult,
    bf16) + PE bf16 matmul scatter-accumulates [sum_u*Xf | sum_u] per
    edge window into PSUM (f32), evacuated to a DRAM edge accumulator
    [20480,129] f32.
  - Gather indices ship compact ([16, n] i16, no 8x lane replication)
    and are replicated to 128 partitions on-device with 3 doubling DVE
    copies; per-call index/scale slices then come from SBUF-resident
    tables (no per-call DMA).
  - ReduceScatter(add, f32) gives each core a 2560-edge shard; phase 2
    normalizes (num/den), un-rotates (Q folded with the score scale),
    applies ELU, matmuls with Wt (+S_features) in bf16, AllGather(bf16)
    -> full Y table [20480,128].
  - Phase 3 mirrors phase 1 with roles swapped: V-sorted windows,
    gather Y rows by E, one-hot scatter into node windows, count via
    ones-column matmul, finalize elu(sum/max(cnt,1)) + X_init per
    window, DMA to the bf16 output shard. Host concatenates + upcasts.
"""

import math

import numpy as np

# hardcoded problem shape (nn_DPHGNNConv_67619965108633)
N_NODES = 100000
N_EDGES = 20000
D = 128
STAR = 64
NSLOPE = 0.2
NCORES = 8

P = 128
NSH = N_NODES // NCORES           # 12500 nodes per core
NWIN3 = (NSH + P - 1) // P        # 98 node windows
NPAD = NWIN3 * P                  # 12544
EPAD = ((N_EDGES + NCORES * P - 1) // (NCORES * P)) * (NCORES * P)  # 20480
NWIN1 = EPAD // P                 # 160 edge windows
ESH = EPAD // NCORES              # 2560 edges per core shard
ETIL = ESH // P                   # 20 tiles per core in phase 2
CPC = 8                           # chunks per dma_gather call (1024 idx max)

_CACHE = {}


def _build(C1, C3):
    import concourse.bass as bass
    import concourse.bacc as bacc
    import concourse.tile as tile
    import concourse.mybir as mybir
    from concourse.masks import make_identity

    f32 = mybir.dt.float32
    bf = mybir.dt.bfloat16
    i16 = mybir.dt.int16
    Alu = mybir.AluOpType
    Act = mybir.ActivationFunctionType

    NCH1 = NWIN1 * C1
    NC1 = NCH1 // CPC             # 160*C1 % 16 == 0
    NCH3 = NWIN3 * C3
    NC3 = (NCH3 + CPC - 1) // CPC

    nc = bacc.Bacc("TRN2", target_bir_lowering=False, debug=False,
                   num_devices=NCORES)
    t_xt = nc.dram_tensor("xt", [P, NPAD], bf, kind="ExternalInput")
    t_st = nc.dram_tensor("st", [STAR, ESH], bf, kind="ExternalInput")
    t_wv = nc.dram_tensor("wv", [P, P], bf, kind="ExternalInput")
    t_wx = nc.dram_tensor("wx", [P, P], bf, kind="ExternalInput")
    t_m = nc.dram_tensor("m", [P, P], bf, kind="ExternalInput")
    t_wt1 = nc.dram_tensor("wt1", [P, P], bf, kind="ExternalInput")
    t_wt2 = nc.dram_tensor("wt2", [STAR, P], bf, kind="ExternalInput")
    t_bv = nc.dram_tensor("bv", [P, P], f32, kind="ExternalInput")
    t_bx = nc.dram_tensor("bx", [P, P], f32, kind="ExternalInput")
    t_bt = nc.dram_tensor("bt", [P, P], f32, kind="ExternalInput")
    t_idx1 = nc.dram_tensor("idx1", [32, NCH1 * 8], i16, kind="ExternalInput")
    t_es1 = nc.dram_tensor("es1", [P, NCH1], f32, kind="ExternalInput")
    t_idx3 = nc.dram_tensor("idx3", [32, NCH3 * 8], i16, kind="ExternalInput")
    t_es3 = nc.dram_tensor("es3", [P, NCH3], f32, kind="ExternalInput")
    t_out = nc.dram_tensor("out", [NSH, P], bf, kind="ExternalOutput")

    with tile.TileContext(nc) as tc:
        with (
            tc.tile_pool(name="const", bufs=1) as cp,
            tc.tile_pool(name="sb", bufs=2) as pool,
            tc.tile_pool(name="psum", bufs=1, space="PSUM") as psp,
            tc.tile_pool(name="dram", bufs=1, space="DRAM") as dp,
        ):
            # constants
            iota_i = cp.tile([P, P], mybir.dt.int32)
            nc.gpsimd.iota(iota_i[:], pattern=[[1, P]], base=0,
                           channel_multiplier=0)
            iota_b = cp.tile([P, P], bf)
            nc.vector.tensor_copy(out=iota_b[:], in_=iota_i[:])
            ones_col = cp.tile([P, 1], bf)
            nc.vector.memset(ones_col[:], 1.0)
            ident = cp.tile([P, P], bf)
            make_identity(nc, ident[:])

            wv = cp.tile([P, P], bf)
            nc.sync.dma_start(out=wv[:], in_=t_wv[:])
            wx = cp.tile([P, P], bf)
            nc.sync.dma_start(out=wx[:], in_=t_wx[:])
            m_t = cp.tile([P, P], bf)
            nc.sync.dma_start(out=m_t[:], in_=t_m[:])
            wt1 = cp.tile([P, P], bf)
            nc.sync.dma_start(out=wt1[:], in_=t_wt1[:])
            wt2 = cp.tile([STAR, P], bf)
            nc.sync.dma_start(out=wt2[:], in_=t_wt2[:])
            bv = cp.tile([P, P], f32)
            nc.sync.dma_start(out=bv[:], in_=t_bv[:])
            bx = cp.tile([P, P], f32)
            nc.sync.dma_start(out=bx[:], in_=t_bx[:])
            bt = cp.tile([P, P], f32)
            nc.sync.dma_start(out=bt[:], in_=t_bt[:])

            xt_s = cp.tile([P, NPAD], bf)
            nc.sync.dma_start(out=xt_s[:], in_=t_xt[:])
            st_s = cp.tile([STAR, ESH], bf)
            nc.sync.dma_start(out=st_s[:], in_=t_st[:])

            # index/scale tables, SBUF-resident for the whole phase
            itab1 = cp.tile([P, NCH1 * 8], i16)
            nc.sync.dma_start(out=itab1[0:32, :], in_=t_idx1[:])
            nc.vector.tensor_copy(out=itab1[32:64, :], in_=itab1[0:32, :])
            nc.vector.tensor_copy(out=itab1[64:128, :], in_=itab1[0:64, :])
            estab1 = cp.tile([P, NCH1], f32)
            nc.sync.dma_start(out=estab1[:], in_=t_es1[:])
            itab3 = cp.tile([P, NCH3 * 8], i16)
            nc.sync.dma_start(out=itab3[0:32, :], in_=t_idx3[:])
            nc.vector.tensor_copy(out=itab3[32:64, :], in_=itab3[0:32, :])
            nc.vector.tensor_copy(out=itab3[64:128, :], in_=itab3[0:64, :])
            estab3 = cp.tile([P, NCH3], f32)
            nc.sync.dma_start(out=estab3[:], in_=t_es3[:])

            xinit = cp.tile([P, NPAD], bf)           # X@Wx^T + bx - 1
            xft = dp.tile([NPAD, P], bf)             # gather table
            eacc = dp.tile([EPAD, P + 1], f32)
            rsout = dp.tile([ESH, P + 1], f32)
            ysh = dp.tile([ESH, P], bf)
            ytab = dp.tile([EPAD, P], bf)

            # ---------------- phase 0: per-shard node features ------------
            for w in range(NWIN3):
                sl = slice(w * P, (w + 1) * P)
                psf = psp.tile([P, P], f32, tag="pa", bufs=4, space="PSUM")
                nc.tensor.matmul(out=psf[:], lhsT=xt_s[:, sl], rhs=wv[:],
                                 start=True, stop=True)
                fsb = pool.tile([P, P], bf, tag="fsb")
                nc.vector.tensor_tensor(out=fsb[:], in0=psf[:], in1=bv[:],
                                        op=Alu.add)
                nc.sync.dma_start(out=xft[sl, :], in_=fsb[:])
                psi = psp.tile([P, P], f32, tag="pa", bufs=4, space="PSUM")
                nc.tensor.matmul(out=psi[:], lhsT=xt_s[:, sl], rhs=wx[:],
                                 start=True, stop=True)
                nc.vector.tensor_tensor(out=xinit[:, sl], in0=psi[:],
                                        in1=bx[:], op=Alu.add)

            # ---------------- phase 1: node -> edge scatter ---------------
            psn = psd = None
            for call in range(NC1):
                gat = pool.tile([P, CPC * P], bf, tag="g1", bufs=3)
                g3 = gat[:].rearrange("p (c e) -> p c e", e=P)
                nc.gpsimd.dma_gather(g3, xft[:, :],
                                     itab1[:, call * CPC * 8:(call + 1) * CPC * 8],
                                     num_idxs=CPC * P, num_idxs_reg=CPC * P,
                                     elem_size=P)
                lr = pool.tile([P, CPC], f32, tag="lr1")
                nc.scalar.activation(out=lr[:], in_=g3[:, :, 0], func=Act.Prelu,
                                     alpha=NSLOPE)
                ui = pool.tile([P, CPC], f32, tag="ui1")
                nc.scalar.activation(out=ui[:], in_=lr[:], func=Act.Exp)
                for c in range(CPC):
                    ch = call * CPC + c
                    w, cl = divmod(ch, C1)
                    oh = pool.tile([P, P], bf, tag="oh1", bufs=4)
                    nc.vector.tensor_scalar(
                        out=oh[:], in0=iota_b[:], scalar1=estab1[:, ch:ch + 1],
                        scalar2=ui[:, c:c + 1], op0=Alu.is_equal, op1=Alu.mult)
                    if cl == 0:
                        psn = psp.tile([P, P], f32, tag="pa", bufs=4,
                                       space="PSUM")
                        psd = psp.tile([P, 1], f32, tag="pb", bufs=2,
                                       space="PSUM")
                    nc.tensor.matmul(out=psn[:], lhsT=oh[:], rhs=g3[:, c, :],
                                     start=(cl == 0), stop=(cl == C1 - 1))
                    nc.tensor.matmul(out=psd[:], lhsT=oh[:], rhs=ones_col[:],
                                     start=(cl == 0), stop=(cl == C1 - 1))
                    if cl == C1 - 1:
                        ev = pool.tile([P, P + 1], f32, tag="ev1", bufs=3)
                        nc.vector.tensor_copy(out=ev[:, :P], in_=psn[:])
                        nc.vector.tensor_copy(out=ev[:, P:P + 1], in_=psd[:])
                        nc.sync.dma_start(out=eacc[w * P:(w + 1) * P, :],
                                          in_=ev[:])

            # ---------------- reduce-scatter edge accumulator -------------
            nc.gpsimd.collective_compute(
                "ReduceScatter", Alu.add,
                replica_groups=[list(range(NCORES))],
                ins=[eacc[0:EPAD // 2, :]], outs=[rsout[0:ESH // 2, :]])
            nc.gpsimd.collective_compute(
                "ReduceScatter", Alu.add,
                replica_groups=[list(range(NCORES))],
                ins=[eacc[EPAD // 2:, :]], outs=[rsout[ESH // 2:, :]])

            # ---------------- phase 2: edge update ------------------------
            for t in range(ETIL):
                sl = slice(t * P, (t + 1) * P)
                rt = pool.tile([P, P + 1], f32, tag="rt2")
                nc.sync.dma_start(out=rt[:], in_=rsout[sl, :])
                den = pool.tile([P, 1], f32, tag="den2")
                nc.vector.tensor_scalar(out=den[:], in0=rt[:, P:P + 1],
                                        scalar1=1e-30, scalar2=None,
                                        op0=Alu.max)
                rec = pool.tile([P, 1], f32, tag="rec2")
                nc.vector.reciprocal(out=rec[:], in_=den[:])
                yt = pool.tile([P, P], bf, tag="yt2")
                nc.vector.tensor_scalar(out=yt[:], in0=rt[:, :P],
                                        scalar1=rec[:], scalar2=None,
                                        op0=Alu.mult)
                pt = psp.tile([P, P], bf, tag="pa", bufs=4, space="PSUM")
                nc.tensor.transpose(out=pt[:], in_=yt[:], identity=ident[:])
                ytT = pool.tile([P, P], bf, tag="ytT2")
                nc.vector.tensor_copy(out=ytT[:], in_=pt[:])
                pr = psp.tile([P, P], f32, tag="pa", bufs=4, space="PSUM")
                nc.tensor.matmul(out=pr[:], lhsT=ytT[:], rhs=m_t[:],
                                 start=True, stop=True)
                # elu(x) = relu(x) + exp(min(x,0)) - 1
                tm = pool.tile([P, P], bf, tag="tm2")
                nc.vector.tensor_scalar(out=tm[:], in0=pr[:], scalar1=0.0,
                                        scalar2=None, op0=Alu.min)
                ex = pool.tile([P, P], bf, tag="ex2")
                nc.scalar.activation(out=ex[:], in_=tm[:], func=Act.Exp)
                rl = pool.tile([P, P], bf, tag="rl2")
                nc.scalar.activation(out=rl[:], in_=pr[:], func=Act.Relu)
                s1 = pool.tile([P, P], bf, tag="s12")
                nc.vector.tensor_tensor(out=s1[:], in0=rl[:], in1=ex[:],
                                        op=Alu.add)
                yv = pool.tile([P, P], bf, tag="yv2")
                nc.vector.tensor_scalar(out=yv[:], in0=s1[:], scalar1=1.0,
                                        scalar2=None, op0=Alu.subtract)
                p2 = psp.tile([P, P], bf, tag="pa", bufs=4, space="PSUM")
                nc.tensor.transpose(out=p2[:], in_=yv[:], identity=ident[:])
                yvT = pool.tile([P, P], bf, tag="yvT2")
                nc.vector.tensor_copy(out=yvT[:], in_=p2[:])
                py = psp.tile([P, P], f32, tag="pa", bufs=4, space="PSUM")
                nc.tensor.matmul(out=py[:], lhsT=yvT[:], rhs=wt1[:],
                                 start=True, stop=False)
                nc.tensor.matmul(out=py[:], lhsT=st_s[:, sl], rhs=wt2[:],
                                 start=False, stop=True)
                yo = pool.tile([P, P], bf, tag="yo2")
                nc.vector.tensor_tensor(out=yo[:], in0=py[:], in1=bt[:],
                                        op=Alu.add)
                nc.sync.dma_start(out=ysh[sl, :], in_=yo[:])

            nc.gpsimd.collective_compute(
                "AllGather", Alu.bypass,
                replica_groups=[list(range(NCORES))],
                ins=[ysh[0:ESH // 2, :]], outs=[ytab[0:EPAD // 2, :]])
            nc.gpsimd.collective_compute(
                "AllGather", Alu.bypass,
                replica_groups=[list(range(NCORES))],
                ins=[ysh[ESH // 2:, :]], outs=[ytab[EPAD // 2:, :]])

            # ---------------- phase 3: edge -> node scatter ---------------
            psn3 = psd3 = None
            for call in range(NC3):
                nch = min(CPC, NCH3 - call * CPC)
                ni = nch * P
                gat = pool.tile([P, CPC * P], bf, tag="g3", bufs=3)
                g3 = gat[:].rearrange("p (c e) -> p c e", e=P)
                nc.gpsimd.dma_gather(g3[:, :nch, :], ytab[:, :],
                                     itab3[:, call * CPC * 8:call * CPC * 8 + nch * 8],
                                     num_idxs=ni, num_idxs_reg=ni,
                                     elem_size=P)
                for c in range(nch):
                    ch = call * CPC + c
                    w, cl = divmod(ch, C3)
                    oh = pool.tile([P, P], bf, tag="oh3", bufs=4)
                    nc.vector.tensor_scalar(
                        out=oh[:], in0=iota_b[:], scalar1=estab3[:, ch:ch + 1],
                        scalar2=None, op0=Alu.is_equal)
                    if cl == 0:
                        psn3 = psp.tile([P, P], f32, tag="pa", bufs=4,
                                        space="PSUM")
                        psd3 = psp.tile([P, 1], f32, tag="pb", bufs=2,
                                        space="PSUM")
                    nc.tensor.matmul(out=psn3[:], lhsT=oh[:], rhs=g3[:, c, :],
                                     start=(cl == 0), stop=(cl == C3 - 1))
                    nc.tensor.matmul(out=psd3[:], lhsT=oh[:], rhs=ones_col[:],
                                     start=(cl == 0), stop=(cl == C3 - 1))
                    if cl == C3 - 1:
                        # finalize node window w
                        cm = pool.tile([P, 1], f32, tag="cm3")
                        nc.vector.tensor_scalar(out=cm[:], in0=psd3[:],
                                                scalar1=1.0, scalar2=None,
                                                op0=Alu.max)
                        rc = pool.tile([P, 1], f32, tag="rc3")
                        nc.vector.reciprocal(out=rc[:], in_=cm[:])
                        xm = pool.tile([P, P], f32, tag="xm3")
                        nc.vector.tensor_scalar(out=xm[:], in0=psn3[:],
                                                scalar1=rc[:], scalar2=None,
                                                op0=Alu.mult)
                        tm = pool.tile([P, P], f32, tag="tm3")
                        nc.vector.tensor_scalar(out=tm[:], in0=xm[:],
                                                scalar1=0.0, scalar2=None,
                                                op0=Alu.min)
                        ex = pool.tile([P, P], f32, tag="ex3")
                        nc.scalar.activation(out=ex[:], in_=tm[:], func=Act.Exp)
                        rl = pool.tile([P, P], f32, tag="rl3")
                        nc.scalar.activation(out=rl[:], in_=xm[:],
                                             func=Act.Relu)
                        s1 = pool.tile([P, P], f32, tag="s13")
                        nc.vector.tensor_tensor(out=s1[:], in0=rl[:],
                                                in1=ex[:], op=Alu.add)
                        ot = pool.tile([P, P], bf, tag="ot3", bufs=3)
                        nc.vector.tensor_tensor(
                            out=ot[:], in0=s1[:],
                            in1=xinit[:, w * P:(w + 1) * P], op=Alu.add)
                        nrow = min(P, NSH - w * P)
                        if nrow > 0:
                            nc.sync.dma_start(
                                out=t_out[w * P:w * P + nrow, :],
                                in_=ot[:nrow, :])

    nc.compile()
    return nc


def kernel(**inputs):
    import sys
    for p in ("/opt/trn_rl_repo", "/opt/pypackages"):
        if p not in sys.path:
            sys.path.insert(0, p)
    import ml_dtypes
    from concourse.bass_utils import run_bass_kernel_spmd

    bf16 = ml_dtypes.bfloat16
    X = np.asarray(inputs["X"], np.float32)
    V = np.asarray(inputs["V"]).astype(np.int64)
    E = np.asarray(inputs["E"]).astype(np.int64)
    S = np.asarray(inputs["S_features"], np.float32)
    Wx_w = np.asarray(inputs["Wx_w"], np.float32)
    Wx_b = np.asarray(inputs["Wx_b"], np.float32)
    Wv_w = np.asarray(inputs["Wv_w"], np.float32)
    Wv_b = np.asarray(inputs["Wv_b"], np.float32)
    a_w = np.asarray(inputs["a_w"], np.float32)
    Wt_w = np.asarray(inputs["Wt_w"], np.float32)
    Wt_b = np.asarray(inputs["Wt_b"], np.float32)

    # ---- weight transforms (host, weights only) ----
    a = a_w[0].astype(np.float64)
    na = float(np.linalg.norm(a))
    au = a / na
    sign = 1.0 if au[0] >= 0 else -1.0
    h = au.copy()
    h[0] += sign
    Q = np.eye(D) - 2.0 * np.outer(h, h) / float(h @ h)  # symmetric, Q@au=-sign*e0
    s_scale = -sign * na

    WVTQS = (Wv_w.T.astype(np.float64) @ Q).astype(np.float32)
    WVTQS[:, 0] *= np.float32(s_scale)
    bvrow = (Wv_b.astype(np.float64) @ Q).astype(np.float32)
    bvrow[0] *= np.float32(s_scale)
    M = Q.copy()
    M[0, :] /= s_scale
    WXT = np.ascontiguousarray(Wx_w.T)
    WT1T = np.ascontiguousarray(Wt_w[:, :D].T)
    WT2T = np.ascontiguousarray(Wt_w[:, D:D + STAR].T)
    BV = np.tile(bvrow, (P, 1)).astype(np.float32)
    BX = np.tile(Wx_b - 1.0, (P, 1)).astype(np.float32)
    BT = np.tile(Wt_b, (P, 1)).astype(np.float32)

    # ---- incidence scheduling (host, index-only, vectorized) ----
    core = V // NSH
    vloc = (V - core * NSH).astype(np.int64)
    nnz = V.shape[0]

    def schedule(key, nwin, idx_val, es_val):
        """key = core*nwin + win. Returns per-core compact idx [8,16,NC*P/16?]
        and es tables plus chunk count C."""
        cnt = np.bincount(key, minlength=NCORES * nwin)
        C = max(1, math.ceil(cnt.max() / P))
        nch = nwin * C
        order = np.argsort(key, kind="stable")
        ks = key[order]
        starts = np.zeros(NCORES * nwin + 1, np.int64)
        np.cumsum(cnt, out=starts[1:])
        rank = np.arange(nnz) - starts[ks]
        win = ks % nwin
        corek = ks // nwin
        pos = win * (C * P) + rank
        idx_slots = np.zeros((NCORES, nch * P), np.int16)
        es_slots = np.full((NCORES, nch * P), -1.0, np.float32)
        idx_slots[corek, pos] = idx_val[order].astype(np.int16)
        es_slots[corek, pos] = es_val[order]
        return C, nch, idx_slots, es_slots

    C1, NCH1, idx1s, es1s = schedule(core * NWIN1 + E // P, NWIN1,
                                     vloc, (E % P).astype(np.float32))
    C3, NCH3, idx3s, es3s = schedule(core * NWIN3 + vloc // P, NWIN3,
                                     E, (vloc % P).astype(np.float32))
    NC1 = NCH1 // CPC
    NC3 = (NCH3 + CPC - 1) // CPC

    def pack_idx(slots, nch, ncall):
        # pad slot array to full calls, then [16, ncall*...] lane-wrap layout
        tot = ncall * CPC * P
        if slots.shape[1] < tot:
            slots = np.concatenate(
                [slots, np.zeros((NCORES, tot - slots.shape[1]), np.int16)],
                axis=1)
        arr = np.ascontiguousarray(
            slots.reshape(NCORES, -1, 16).transpose(0, 2, 1))[:, :, :nch * 8]
        return np.tile(arr, (1, 2, 1))  # 32 partitions (verifier alignment)

    def pack_es(slots, nch):
        # [128, nch] with column ch = chunk, row p = slot within chunk
        return np.ascontiguousarray(
            slots.reshape(NCORES, nch, P).transpose(0, 2, 1)).astype(np.float32)

    idx1 = pack_idx(idx1s, NCH1, NC1)
    es1 = pack_es(es1s, NCH1)
    idx3 = pack_idx(idx3s, NCH3, NC3)
    es3 = pack_es(es3s, NCH3)

    WVb = WVTQS.astype(bf16)
    WXb = WXT.astype(bf16)
    Mb = M.astype(bf16)
    WT1b = WT1T.astype(bf16)
    WT2b = WT2T.astype(bf16)

    in_maps = []
    for k in range(NCORES):
        xt = np.zeros((P, NPAD), bf16)
        xt[:, :NSH] = X[k * NSH:(k + 1) * NSH].T
        st = np.zeros((STAR, ESH), bf16)
        HS = ESH // 2
        for hi, base in enumerate((k * HS, EPAD // 2 + k * HS)):
            r0, r1 = base, min(base + HS, N_EDGES)
            if r1 > r0:
                st[:, hi * HS:hi * HS + (r1 - r0)] = S[r0:r1].T
        in_maps.append({
            "xt": xt, "st": st, "wv": WVb, "wx": WXb, "m": Mb,
            "wt1": WT1b, "wt2": WT2b, "bv": BV, "bx": BX, "bt": BT,
            "idx1": idx1[k], "es1": es1[k], "idx3": idx3[k], "es3": es3[k],
        })

    key = (C1, C3)
    if key not in _CACHE:
        _CACHE[key] = _build(C1, C3)
    nc = _CACHE[key]

    res = run_bass_kernel_spmd(nc, in_maps, core_ids=list(range(NCORES)))
    out = np.concatenate(
        [res.results[k]["out"].astype(np.float32) for k in range(NCORES)],
        axis=0)
    return out
